# revision 25
# baseline (speedup 1.0000x reference)
"""Trainium2 Bass kernel for BidirectionalAttention — fp8 DoubleRow, DMA-xbar
transpose version.

Reference computation (per batch element n, D=1024, S=T=2048):
    L = tanh(lhs @ W_lhs.T)              # (S, D)
    R = tanh(rhs @ W_rhs.T)              # (T, D)
    scores = L @ R.T                     # (S, T)
    A1 = softmax(scores / 32, axis=1)    # over t
    A2 = softmax(scores / 32, axis=0)    # over s
    out_lhs = [lhs, A1 @ rhs]            # (S, 2D)
    out_rhs = [rhs, A2.T @ lhs]          # (T, 2D)

Sharding: data-parallel over batch N=8 across the 8 NeuronCores; each core
computes one batch element end-to-end (no collectives).

Kernel strategy (per core):
  - ALL four big matmuls (projections, scores, C1, C2) run as fp8e4
    DoubleRow matmuls (contraction 256 per instruction, 2x bf16 FLOP rate).
    That puts the PE matmul floor at ~218us; everything else is arranged to
    keep the PE at that floor.
  - The e->eT transposes needed by C1 (contraction over t) do NOT run on the
    PE (the previous version spent ~27us of PE time on 256 identity-matmul
    transposes). Instead each s-block of exp(scores) is viewed as uint16
    (pairs of adjacent-t fp8 values), pushed through the DMA xbar transpose
    (16x128 tiles, ~14ns/tile on an otherwise-idle DMA queue), and byte-
    de-interleaved by the GpSimd engine into a standard DoubleRow stationary
    layout. The packed pair (t=2u, t=2u+1) lands on partition u%128, so the
    host ships rhs in a matching interleaved layout
    rhs_dr[p, c, b, d] = rhs[256c + 2p + b, d] and the DoubleRow contraction
    pairing stays consistent end-to-end.
  - Column sums for the axis=0 softmax are accumulated on the DVE by
    reducing the de-interleaved eT panels over s (t sits on partitions
    there); the interleaved per-partition layout is straightened out by two
    tiny SBUF->SBUF shuffle DMAs at the D->E boundary. The last two s-blocks
    are added via one ones-matmul per t-block in phase E (issued FIRST per
    t-block so the reciprocal is ready before the C2 psums land).
  - Startup DMAs are spread over three queues (weights-c0 on ACT, the
    transposed inputs on SP, weights-c1 + naturals on Pool) so the PE's
    projection stream is never starved.
  - The raw input halves of both outputs are concatenated on the host;
    the device only computes and returns the two context halves.
"""

import math
import os
import sys
from contextlib import ExitStack

import numpy as np

sys.path.insert(0, "/opt/trn_rl_repo")

import ml_dtypes

import concourse.bass as bass
import concourse.tile as tile
from concourse import bacc, mybir

D = 1024
S = 2048
P = 128
ND = D // P   # 8 chunks along d/e
NS = S // P   # 16 blocks along s/t
NC = D // 256  # 4 DoubleRow chunks per 1024 contraction
N_CORES = 8
SCALE = 1.0 / math.sqrt(D)   # 1/32
WSCALE = 32.0                # host multiplies W by this before fp8 quant

FP32 = mybir.dt.float32
FP8 = mybir.dt.float8e4
U16 = mybir.dt.uint16
DR = mybir.MatmulPerfMode.DoubleRow

# set by kernel() when profiling is enabled via KERNEL_TRACE=1
last_exec_time_ns = None
last_results = None


def _build_body(ctx: ExitStack, tc: tile.TileContext, lhsT8, rhsT8, lhs8,
                rhs_dr8, wl8, wr8, ctx_l, ctx_r):
    nc = tc.nc

    singles = ctx.enter_context(tc.tile_pool(name="singles", bufs=1))

    # PSUM pools: 6 + 2 banks = all 8 (pp_c1 holds the two C1 psums per
    # block; bufs=1 is enough rotation since block j's C1 psums only WAR on
    # the ts_muls of block j-2, one pipeline stage earlier)
    pp6 = ctx.enter_context(tc.tile_pool(name="pp6", bufs=6, space="PSUM"))
    pp_c1 = ctx.enter_context(tc.tile_pool(name="pp_c1", bufs=1, space="PSUM"))

    stats = ctx.enter_context(tc.tile_pool(name="stats", bufs=4))
    outp = ctx.enter_context(tc.tile_pool(name="outp", bufs=3))
    # packed (uint16) transposed tiles and de-interleaved fp8 panels
    eTpk = ctx.enter_context(tc.tile_pool(name="eTpk", bufs=2))
    eTp = ctx.enter_context(tc.tile_pool(name="eTp", bufs=3))

    # persistent compute tensors
    projout = ctx.enter_context(tc.tile_pool(name="projout", bufs=1))
    RT = projout.tile([P, ND, S], FP8, tag="RT")   # [e%128, ec, t]
    LT = projout.tile([P, ND, S], FP8, tag="LT")   # [e%128, ec, s]
    natp = ctx.enter_context(tc.tile_pool(name="natp", bufs=1))
    # interleaved rhs for C1: rhs_nat2[p, c, b, d] = rhs[256c + 2p + b, d]
    rhs_nat2 = natp.tile([P, S // 256, 2, D], FP8, tag="rhs_nat2")
    lhs_nat = natp.tile([P, NS, D], FP8, tag="lhs_nat")  # [s%128, sc, d]
    ep = ctx.enter_context(tc.tile_pool(name="ep", bufs=1))
    e_full = ep.tile([P, NS, S], FP8, tag="e_full")      # [s%128, sb, t]
    # per-s-block partial column sums from the DVE reduces of the eT panels:
    # csacc[p, b, uc, j] = sum_{s in block j} e[s, t=256uc+2p+b]
    csacc = ep.tile([P, 2, ND, NS], FP32, tag="csacc")
    # straightened column sums (after the shuffle DMAs): colsum_ev[p', uc] is
    # the blocks-0..13 column sum for t = 256uc + p' (even t-blocks), _od for
    # t = 256uc + 128 + p' (odd t-blocks)
    csum_t = singles.tile([P, 2, ND, 1], FP32, tag="csum_t")  # [p, b, uc]
    colsum_ev = singles.tile([P, ND], FP32, tag="colsum_ev")
    colsum_od = singles.tile([P, ND], FP32, tag="colsum_od")

    # ---------------- Phase A: load weights + transposed inputs -----------
    phase_b = ExitStack()
    wpool = phase_b.enter_context(tc.tile_pool(name="wpool", bufs=1))
    wr_sb = wpool.tile([P, ND, D], FP8, tag="wr")   # [d%128, dc, e]
    wl_sb = wpool.tile([P, ND, D], FP8, tag="wl")
    tpool = phase_b.enter_context(tc.tile_pool(name="tpool", bufs=1))
    rT_sb = tpool.tile([P, ND, S], FP8, tag="rT")   # [d%128, dc, t]
    lT_sb = tpool.tile([P, ND, S], FP8, tag="lT")   # [d%128, dc, s]

    # All inputs are DMA'd at dc-PAIR granularity directly into the tiles the
    # matmuls consume: every DoubleRow operand slice [:, 2i:2i+2, ...] then
    # depends on exactly ONE DMA, so each matmul/ldweights needs at most one
    # DMA-queue semaphore wait (plus its PSUM WAR).
    def dma_pair(src, dst, i, col0=0, col1=None, eng=None):
        cols = src.shape[-1] if col1 is None else col1 - col0
        (eng or nc.sync).dma_start(
            out=dst[:, 2 * i:2 * i + 2, col0:col0 + cols],
            in_=src[i * 2 * P:(i + 1) * 2 * P, col0:col0 + cols]
                .rearrange("(two p) f -> p two f", p=P))

    # Startup DMA plan (same interleave the baseline measured fastest):
    # weights' first halves ride the ACT queue (idle until the first tanh),
    # everything streamed rides SP in the projections' consumption order;
    # only the phase-D/E natural-layout inputs move to the Pool queue.
    for wi, (w8, wsb, t8, tsb) in enumerate(((wr8, wr_sb, rhsT8, rT_sb),
                                             (wl8, wl_sb, lhsT8, lT_sb))):
        for i in range(ND // 2):
            dma_pair(w8, wsb, i, 0, 512,
                     eng=nc.scalar if wi == 0 else nc.sync)
            dma_pair(t8, tsb, i, 0, 512)
        for i in range(ND // 2):
            dma_pair(w8, wsb, i, 512, D)
            dma_pair(t8, tsb, i, 512, 1024)
        for c in (1024, 1536):
            for i in range(ND // 2):
                dma_pair(t8, tsb, i, c, c + 512)
    # interleaved rhs + natural lhs stream in on the Pool queue during B
    for c in range(S // 256):
        nc.gpsimd.dma_start(out=rhs_nat2[:, c, :, :], in_=rhs_dr8[:, c, :, :])
    for i in range(NS // 2):
        dma_pair(lhs8, lhs_nat, i, eng=nc.gpsimd)

    # ---------------- Phase B: projections (DoubleRow) --------------------
    # R^T = tanh((32W_r)^T@rhs^T / 32), L^T likewise. Stationary = weight
    # block [d-pair, e-block(128)], moving = input^T [d-pair, 512 cols].
    # q-major: consumption of each input column chunk is spread over all
    # eight eb blocks, matching the DMA arrival order above.
    for w_sb, src, dst in ((wr_sb, rT_sb, RT), (wl_sb, lT_sb, LT)):
        for q in range(4):
            for eb in range(ND):
                ps = pp6.tile([P, 512], FP32, tag="pp6")
                for dcp in range(4):
                    nc.tensor.matmul(
                        ps,
                        lhsT=w_sb[:, 2 * dcp:2 * dcp + 2, eb * P:(eb + 1) * P],
                        rhs=src[:, 2 * dcp:2 * dcp + 2, q * 512:(q + 1) * 512],
                        start=(dcp == 0), stop=(dcp == 3), perf_mode=DR)
                nc.scalar.activation(
                    out=dst[:, eb, q * 512:(q + 1) * 512], in_=ps,
                    func=mybir.ActivationFunctionType.Tanh, scale=1.0 / WSCALE)

    phase_b.close()

    # ---------------- Phase D: scores -> exp -> xbar eT -> C1 -------------
    # Software-pipelined by TWO s-blocks: while the PE runs scores(j) and
    # C1(j-2), block j-1 flows through exp (ACT) -> xbar transpose (SP DMA)
    # -> de-interleave (Pool) with ~7us of slack before C1(j-1) needs it.
    carry = {}

    def scores_mms(j):
        ps = [pp6.tile([P, 512], FP32, tag="pp6", name=f"pp6_{qi}")
              for qi in range(4)]
        for ecp in range(4):
            for tq in range(4):
                nc.tensor.matmul(
                    ps[tq],
                    lhsT=LT[:, 2 * ecp:2 * ecp + 2, j * P:(j + 1) * P],
                    rhs=RT[:, 2 * ecp:2 * ecp + 2, tq * 512:(tq + 1) * 512],
                    start=(ecp == 0), stop=(ecp == 3), perf_mode=DR)
        return ps

    def scores_exp(j, ps):
        # no accum_out: the ACTIVATION_READ_ACCUMULATOR after each exp costs
        # ~285ns of serial ACT time and couples the scores psum WAR chain to
        # it; the row sums come from one DVE reduce over e_full instead
        for tq in range(4):
            nc.scalar.activation(
                out=e_full[:, j, tq * 512:(tq + 1) * 512], in_=ps[tq],
                func=mybir.ActivationFunctionType.Exp, scale=SCALE)

    def rowsum_recip(j):
        rowsum = stats.tile([P, 1], FP32, tag="rs")
        nc.vector.reduce_sum(out=rowsum, in_=e_full[:, j, :],
                             axis=mybir.AxisListType.X)
        rrec = stats.tile([P, 1], FP32, tag="rrec")
        nc.vector.reciprocal(out=rrec, in_=rowsum)
        carry[j] = rrec

    def transposes(j):
        # e_full[:, j, :] viewed as uint16 pairs -> packed transposed tile
        # eT16[p, uc, s]: the uint16 at (p, uc, s) is the fp8 pair
        # (e[s, 256uc+2p], e[s, 256uc+2p+1]). Two halves so the first can
        # launch as soon as the first two exp quadrants are done.
        eT16 = eTpk.tile([P, ND, P], U16, tag="eT16")
        e16 = e_full[:, j, :].bitcast(U16)
        nc.sync.dma_start_transpose(out=eT16[:, 0:ND // 2, :],
                                    in_=e16[:, 0:S // 4])
        nc.sync.dma_start_transpose(out=eT16[:, ND // 2:ND, :],
                                    in_=e16[:, S // 4:S // 2])
        return eT16

    def deint(j, eT16):
        # Byte de-interleave into the standard DoubleRow stationary layout:
        # eT_panel[p, uc, b, s] = e[s, 256uc+2p+b]. One half on ACT, one on
        # DVE (~1.1us each): the Pool engine runs this strided byte copy ~5x
        # slower, and putting both on ACT made ACT the critical engine.
        eT_panel = eTp.tile([P, ND, 2, P], FP8, tag="eTp")
        for h, eng in ((0, nc.scalar), (1, nc.vector)):
            sl = slice(h * (ND // 2), (h + 1) * (ND // 2))
            src = eT16[:, sl, :].bitcast(FP8).rearrange(
                "p uc (s two) -> p uc two s", two=2)
            if eng is nc.scalar:
                eng.copy(out=eT_panel[:, sl, :, :], in_=src)
            else:
                eng.tensor_copy(out=eT_panel[:, sl, :, :], in_=src)
        return eT_panel

    def ctx1_mms(i, eT_panel):
        rrec = carry.pop(i)
        # C1: ctx_l[i-block] = (e @ rhs) * rrec; uc-outer so the two q psums
        # share each stationary load
        qs = [pp_c1.tile([P, 512], FP32, tag="c1", name=f"c1_{qi}")
              for qi in range(2)]
        for uc in range(S // 256):
            for q in range(2):
                nc.tensor.matmul(
                    qs[q],
                    lhsT=eT_panel[:, uc, :, :],
                    rhs=rhs_nat2[:, uc, :, q * 512:(q + 1) * 512],
                    start=(uc == 0), stop=(uc == S // 256 - 1), perf_mode=DR)
        osb = outp.tile([P, D], FP32, tag="osb")
        for q in range(2):
            nc.vector.tensor_scalar_mul(
                out=osb[:, q * 512:(q + 1) * 512], in0=qs[q], scalar1=rrec)
            nc.sync.dma_start(
                out=ctx_l[i * P:(i + 1) * P, q * 512:(q + 1) * 512],
                in_=osb[:, q * 512:(q + 1) * 512])

    def csacc_reduce(i, eT_panel):
        # partial column sums on the DVE, issued last in the DVE queue so
        # they fill idle time; out csacc[p, b, uc, i]
        nc.vector.reduce_sum(
            out=csacc[:, :, :, i:i + 1],
            in_=eT_panel[:].rearrange("p uc two s -> p two uc s"),
            axis=mybir.AxisListType.X)

    pipe = {}
    tpk = {}
    for j in range(NS + 2):
        if j < NS:
            ps = scores_mms(j)
        if 1 <= j <= NS:
            # ACT: de-interleave block j-1 first (inputs landed last iter),
            # so the copies never make the ACT queue idle-wait mid-iteration
            pipe[j - 1] = deint(j - 1, tpk.pop(j - 1))
        if 1 <= j <= NS:
            # DVE: independent work first (row sums of j-1, its column-sum
            # partials), the PE-gated ts_muls of j-2 last
            rowsum_recip(j - 1)
        if j < NS:
            scores_exp(j, ps)
            tpk[j] = transposes(j)
        if 1 <= j <= NS:
            csacc_reduce(j - 1, pipe[j - 1])
        if j >= 2:
            ctx1_mms(j - 2, pipe.pop(j - 2))

    # ---------------- D->E boundary: straighten the column sums -----------
    # csum_t[p, b, uc] = sum over all 16 blocks of csacc; then four tiny
    # shuffle DMAs map (p, b) -> partition p'=2p+b for even (p<64) / odd
    # (p>=64) t-blocks: colsum_ev[p', uc] = column sum for t-block 2uc at
    # t%128 = p'. Ready ~2us into phase E; the first ts_mul consumer has
    # ~10us of phase-E pipeline ahead of it.
    nc.vector.reduce_sum(out=csum_t[:], in_=csacc[:],
                         axis=mybir.AxisListType.X)
    for dst, p0 in ((colsum_ev, 0), (colsum_od, P // 2)):
        half = dst[:].rearrange("(p two) uc -> p two uc", two=2)
        for b in range(2):
            nc.gpsimd.dma_start(out=half[:, b, :],
                                in_=csum_t[p0:p0 + P // 2, b, :, 0])

    # ---------------- Phase E: C2 (column softmax context) ----------------
    # The last t-block runs q-major so its first half can drain while the
    # second half multiplies.
    for tb in range(NS):
        qs = [pp6.tile([P, 512], FP32, tag="pp6", name=f"pp6_{qi}")
              for qi in range(2)]
        if tb < NS - 1:
            for scp in range(ND):
                lw = e_full[:, 2 * scp:2 * scp + 2, tb * P:(tb + 1) * P]
                for q in range(2):
                    nc.tensor.matmul(
                        qs[q], lhsT=lw,
                        rhs=lhs_nat[:, 2 * scp:2 * scp + 2,
                                    q * 512:(q + 1) * 512],
                        start=(scp == 0), stop=(scp == ND - 1), perf_mode=DR)
        else:
            for q in range(2):
                for scp in range(ND):
                    nc.tensor.matmul(
                        qs[q],
                        lhsT=e_full[:, 2 * scp:2 * scp + 2,
                                    tb * P:(tb + 1) * P],
                        rhs=lhs_nat[:, 2 * scp:2 * scp + 2,
                                    q * 512:(q + 1) * 512],
                        start=(scp == 0), stop=(scp == ND - 1), perf_mode=DR)
        colsum_part = colsum_ev if tb % 2 == 0 else colsum_od
        crec = stats.tile([P, 1], FP32, tag="crec")
        nc.vector.reciprocal(out=crec,
                             in_=colsum_part[:, tb // 2:tb // 2 + 1])
        osb = outp.tile([P, D], FP32, tag="osb")
        for q in range(2):
            nc.vector.tensor_scalar_mul(
                out=osb[:, q * 512:(q + 1) * 512], in0=qs[q], scalar1=crec)
            nc.sync.dma_start(
                out=ctx_r[tb * P:(tb + 1) * P, q * 512:(q + 1) * 512],
                in_=osb[:, q * 512:(q + 1) * 512])


def build_bass():
    nc = bacc.Bacc()
    lhsT8 = nc.declare_dram_parameter("lhsT8", [D, S], FP8, isOutput=False)
    rhsT8 = nc.declare_dram_parameter("rhsT8", [D, S], FP8, isOutput=False)
    lhs8 = nc.declare_dram_parameter("lhs8", [S, D], FP8, isOutput=False)
    rhs_dr8 = nc.declare_dram_parameter("rhs_dr8", [P, S // 256, 2, D], FP8,
                                        isOutput=False)
    wl8 = nc.declare_dram_parameter("wl8", [D, D], FP8, isOutput=False)
    wr8 = nc.declare_dram_parameter("wr8", [D, D], FP8, isOutput=False)
    ctx_l = nc.declare_dram_parameter("ctx_l", [S, D], FP32, isOutput=True)
    ctx_r = nc.declare_dram_parameter("ctx_r", [S, D], FP32, isOutput=True)
    with tile.TileContext(nc) as tc:
        with ExitStack() as ctx:
            _build_body(ctx, tc, lhsT8[:], rhsT8[:], lhs8[:], rhs_dr8[:],
                        wl8[:], wr8[:], ctx_l[:], ctx_r[:])
    nc.compile()
    return nc


def _profiled_run(nc, in_maps):
    """Run via PJRT with NTFF profiling of core 0; returns (results, info)."""
    import glob
    import tempfile

    from concourse import bass2jax

    try:
        from trn_agent_boot.trn_boot import _ntff_profile_via_ctypes
        hook = _ntff_profile_via_ctypes("/opt/axon/libaxon_pjrt.so")
    except Exception as e:
        print(f"[kernel] NTFF hook unavailable ({e}); running untraced",
              file=sys.stderr)
        hook = None
    if hook is None:
        return bass2jax.run_bass_via_pjrt(nc, in_maps, n_cores=N_CORES), None

    tmpdir = tempfile.mkdtemp(prefix="bass_ntff_")
    with hook(tmpdir, [0]):
        results = bass2jax.run_bass_via_pjrt(nc, in_maps, n_cores=N_CORES)

    ntffs = glob.glob(os.path.join(tmpdir, "*_body*.ntff"))
    if not ntffs:
        print(f"[kernel] no NTFFs in {tmpdir}: {os.listdir(tmpdir)}",
              file=sys.stderr)
        return results, None
    import gauge.profiler
    from concourse._compat import FishPath

    profile = gauge.profiler.Profile(
        profile_path=FishPath(tmpdir),
        kernel_dev_mode=True,
        profile_on_exit=False,
        bass_kernel=nc.m,
        offline_processing=True,
        fname="*_body*",
    )
    try:
        pres = profile.to_perfetto(model_index=(0,))
        if pres:
            return results, (pres[0].exec_time_ns, pres[0].trace_path, tmpdir,
                             pres[0].insts)
    except Exception as e:
        print(f"[kernel] perfetto conversion failed: {e}", file=sys.stderr)
    return results, None


def kernel(lhs, rhs, W_lhs, W_rhs):
    """Full inputs in, full outputs out. Shards batch across 8 cores."""
    global last_exec_time_ns, last_results
    from concourse import bass2jax

    f8 = ml_dtypes.float8_e4m3
    lhs = np.ascontiguousarray(np.asarray(lhs, dtype=np.float32))
    rhs = np.ascontiguousarray(np.asarray(rhs, dtype=np.float32))
    lhs8 = lhs.astype(f8)
    rhs8 = rhs.astype(f8)
    lhsT8 = np.ascontiguousarray(lhs.transpose(0, 2, 1)).astype(f8)
    rhsT8 = np.ascontiguousarray(rhs.transpose(0, 2, 1)).astype(f8)
    # interleaved DoubleRow layout for C1's moving operand:
    # rhs_dr8[n, p, c, b, d] = rhs[n, 256c + 2p + b, d]
    rhs_dr8 = np.ascontiguousarray(
        rhs8.reshape(rhs8.shape[0], S // 256, P, 2, D).transpose(0, 2, 1, 3, 4))
    wl8 = np.ascontiguousarray(
        np.asarray(W_lhs, dtype=np.float32).T * WSCALE).astype(f8)
    wr8 = np.ascontiguousarray(
        np.asarray(W_rhs, dtype=np.float32).T * WSCALE).astype(f8)

    nc = build_bass()
    in_maps = [
        {"lhsT8": lhsT8[i], "rhsT8": rhsT8[i], "lhs8": lhs8[i],
         "rhs_dr8": rhs_dr8[i], "wl8": wl8, "wr8": wr8}
        for i in range(N_CORES)
    ]
    if os.environ.get("KERNEL_TRACE", "0") == "1":
        results, info = _profiled_run(nc, in_maps)
        if info is not None:
            last_exec_time_ns = info[0]
            last_results = info
    else:
        results = bass2jax.run_bass_via_pjrt(nc, in_maps, n_cores=N_CORES)
    ctx_l = np.stack([np.asarray(results[i]["ctx_l"]) for i in range(N_CORES)])
    ctx_r = np.stack([np.asarray(results[i]["ctx_r"]) for i in range(N_CORES)])
    out_lhs = np.concatenate([lhs, ctx_l], axis=2)
    out_rhs = np.concatenate([rhs, ctx_r], axis=2)
    return out_lhs, out_rhs


# revision 27
# speedup vs baseline: 1.0669x; 1.0669x over previous
"""Trainium2 Bass kernel for BidirectionalAttention — fp8 DoubleRow, DMA-xbar
transpose version.

Reference computation (per batch element n, D=1024, S=T=2048):
    L = tanh(lhs @ W_lhs.T)              # (S, D)
    R = tanh(rhs @ W_rhs.T)              # (T, D)
    scores = L @ R.T                     # (S, T)
    A1 = softmax(scores / 32, axis=1)    # over t
    A2 = softmax(scores / 32, axis=0)    # over s
    out_lhs = [lhs, A1 @ rhs]            # (S, 2D)
    out_rhs = [rhs, A2.T @ lhs]          # (T, 2D)

Sharding: data-parallel over batch N=8 across the 8 NeuronCores; each core
computes one batch element end-to-end (no collectives).

Kernel strategy (per core):
  - ALL four big matmuls (projections, scores, C1, C2) run as fp8e4
    DoubleRow matmuls (contraction 256 per instruction, 2x bf16 FLOP rate).
    That puts the PE matmul floor at ~218us; everything else is arranged to
    keep the PE at that floor.
  - The e->eT transposes needed by C1 (contraction over t) do NOT run on the
    PE (the previous version spent ~27us of PE time on 256 identity-matmul
    transposes). Instead each s-block of exp(scores) is viewed as uint16
    (pairs of adjacent-t fp8 values), pushed through the DMA xbar transpose
    (16x128 tiles, ~14ns/tile on an otherwise-idle DMA queue), and byte-
    de-interleaved by the GpSimd engine into a standard DoubleRow stationary
    layout. The packed pair (t=2u, t=2u+1) lands on partition u%128, so the
    host ships rhs in a matching interleaved layout
    rhs_dr[p, c, b, d] = rhs[256c + 2p + b, d] and the DoubleRow contraction
    pairing stays consistent end-to-end.
  - Column sums for the axis=0 softmax are accumulated on the DVE by
    reducing the de-interleaved eT panels over s (t sits on partitions
    there); the interleaved per-partition layout is straightened out by two
    tiny SBUF->SBUF shuffle DMAs at the D->E boundary. The last two s-blocks
    are added via one ones-matmul per t-block in phase E (issued FIRST per
    t-block so the reciprocal is ready before the C2 psums land).
  - Startup DMAs are spread over three queues (weights-c0 on ACT, the
    transposed inputs on SP, weights-c1 + naturals on Pool) so the PE's
    projection stream is never starved.
  - The raw input halves of both outputs are concatenated on the host;
    the device only computes and returns the two context halves.
"""

import math
import os
import sys
from contextlib import ExitStack

import numpy as np

sys.path.insert(0, "/opt/trn_rl_repo")

import ml_dtypes

import concourse.bass as bass
import concourse.tile as tile
from concourse import bacc, mybir

D = 1024
S = 2048
P = 128
ND = D // P   # 8 chunks along d/e
NS = S // P   # 16 blocks along s/t
NC = D // 256  # 4 DoubleRow chunks per 1024 contraction
N_CORES = 8
SCALE = 1.0 / math.sqrt(D)   # 1/32
WSCALE = 32.0                # host multiplies W by this before fp8 quant

FP32 = mybir.dt.float32
FP8 = mybir.dt.float8e4
U16 = mybir.dt.uint16
DR = mybir.MatmulPerfMode.DoubleRow

# set by kernel() when profiling is enabled via KERNEL_TRACE=1
last_exec_time_ns = None
last_results = None


def _build_body(ctx: ExitStack, tc: tile.TileContext, lhsT8, rhsT8, lhs8,
                rhs_dr8, wl8, wr8, ctx_l, ctx_r):
    nc = tc.nc

    singles = ctx.enter_context(tc.tile_pool(name="singles", bufs=1))

    # PSUM pools: 6 + 2 banks = all 8 (pp_c1 holds the two C1 psums per
    # block; bufs=1 is enough rotation since block j's C1 psums only WAR on
    # the ts_muls of block j-2, one pipeline stage earlier)
    pp6 = ctx.enter_context(tc.tile_pool(name="pp6", bufs=6, space="PSUM"))
    pp_c1 = ctx.enter_context(tc.tile_pool(name="pp_c1", bufs=1, space="PSUM"))

    stats = ctx.enter_context(tc.tile_pool(name="stats", bufs=4))
    outp = ctx.enter_context(tc.tile_pool(name="outp", bufs=3))
    # packed (uint16) transposed tiles and de-interleaved fp8 panels
    eTpk = ctx.enter_context(tc.tile_pool(name="eTpk", bufs=2))
    eTp = ctx.enter_context(tc.tile_pool(name="eTp", bufs=3))

    # persistent compute tensors
    projout = ctx.enter_context(tc.tile_pool(name="projout", bufs=1))
    RT = projout.tile([P, ND, S], FP8, tag="RT")   # [e%128, ec, t]
    LT = projout.tile([P, ND, S], FP8, tag="LT")   # [e%128, ec, s]
    natp = ctx.enter_context(tc.tile_pool(name="natp", bufs=1))
    # interleaved rhs for C1: rhs_nat2[p, c, b, d] = rhs[256c + 2p + b, d]
    rhs_nat2 = natp.tile([P, S // 256, 2, D], FP8, tag="rhs_nat2")
    lhs_nat = natp.tile([P, NS, D], FP8, tag="lhs_nat")  # [s%128, sc, d]
    ep = ctx.enter_context(tc.tile_pool(name="ep", bufs=1))
    e_full = ep.tile([P, NS, S], FP8, tag="e_full")      # [s%128, sb, t]
    # per-s-block partial column sums from the DVE reduces of the eT panels:
    # csacc[p, b, uc, j] = sum_{s in block j} e[s, t=256uc+2p+b]
    csacc = ep.tile([P, 2, ND, NS], FP32, tag="csacc")
    # straightened column sums (after the shuffle DMAs): colsum_ev[p', uc] is
    # the blocks-0..13 column sum for t = 256uc + p' (even t-blocks), _od for
    # t = 256uc + 128 + p' (odd t-blocks)
    csum_t = singles.tile([P, 2, ND, 1], FP32, tag="csum_t")  # [p, b, uc]
    colsum_ev = singles.tile([P, ND], FP32, tag="colsum_ev")
    colsum_od = singles.tile([P, ND], FP32, tag="colsum_od")

    # ---------------- Phase A: load weights + transposed inputs -----------
    phase_b = ExitStack()
    wpool = phase_b.enter_context(tc.tile_pool(name="wpool", bufs=1))
    wr_sb = wpool.tile([P, ND, D], FP8, tag="wr")   # [d%128, dc, e]
    wl_sb = wpool.tile([P, ND, D], FP8, tag="wl")
    tpool = phase_b.enter_context(tc.tile_pool(name="tpool", bufs=1))
    rT_sb = tpool.tile([P, ND, S], FP8, tag="rT")   # [d%128, dc, t]
    lT_sb = tpool.tile([P, ND, S], FP8, tag="lT")   # [d%128, dc, s]

    # All inputs are DMA'd at dc-PAIR granularity directly into the tiles the
    # matmuls consume: every DoubleRow operand slice [:, 2i:2i+2, ...] then
    # depends on exactly ONE DMA, so each matmul/ldweights needs at most one
    # DMA-queue semaphore wait (plus its PSUM WAR).
    def dma_pair(src, dst, i, col0=0, col1=None, eng=None):
        cols = src.shape[-1] if col1 is None else col1 - col0
        (eng or nc.sync).dma_start(
            out=dst[:, 2 * i:2 * i + 2, col0:col0 + cols],
            in_=src[i * 2 * P:(i + 1) * 2 * P, col0:col0 + cols]
                .rearrange("(two p) f -> p two f", p=P))

    # Startup DMA plan (same interleave the baseline measured fastest):
    # weights' first halves ride the ACT queue (idle until the first tanh),
    # everything streamed rides SP in the projections' consumption order;
    # only the phase-D/E natural-layout inputs move to the Pool queue.
    for wi, (w8, wsb, t8, tsb) in enumerate(((wr8, wr_sb, rhsT8, rT_sb),
                                             (wl8, wl_sb, lhsT8, lT_sb))):
        for i in range(ND // 2):
            dma_pair(w8, wsb, i, 0, 512,
                     eng=nc.scalar if wi == 0 else nc.sync)
            dma_pair(t8, tsb, i, 0, 512)
        for i in range(ND // 2):
            dma_pair(w8, wsb, i, 512, D)
            dma_pair(t8, tsb, i, 512, 1024)
        for c in (1024, 1536):
            for i in range(ND // 2):
                dma_pair(t8, tsb, i, c, c + 512)
    # interleaved rhs + natural lhs stream in at the SP queue tail (issuing
    # them early on the Pool queue steals HBM bandwidth from the startup-
    # critical weight/input chunks above - measured +12us on phase B)
    for c in range(S // 256):
        nc.sync.dma_start(out=rhs_nat2[:, c, :, :], in_=rhs_dr8[:, c, :, :])
    for i in range(NS // 2):
        dma_pair(lhs8, lhs_nat, i)

    # ---------------- Phase B: projections (DoubleRow) --------------------
    # R^T = tanh((32W_r)^T@rhs^T / 32), L^T likewise. Stationary = weight
    # block [d-pair, e-block(128)], moving = input^T [d-pair, 512 cols].
    # q-major: consumption of each input column chunk is spread over all
    # eight eb blocks, matching the DMA arrival order above.
    for w_sb, src, dst in ((wr_sb, rT_sb, RT), (wl_sb, lT_sb, LT)):
        for q in range(4):
            for eb in range(ND):
                ps = pp6.tile([P, 512], FP32, tag="pp6")
                for dcp in range(4):
                    nc.tensor.matmul(
                        ps,
                        lhsT=w_sb[:, 2 * dcp:2 * dcp + 2, eb * P:(eb + 1) * P],
                        rhs=src[:, 2 * dcp:2 * dcp + 2, q * 512:(q + 1) * 512],
                        start=(dcp == 0), stop=(dcp == 3), perf_mode=DR)
                nc.scalar.activation(
                    out=dst[:, eb, q * 512:(q + 1) * 512], in_=ps,
                    func=mybir.ActivationFunctionType.Tanh, scale=1.0 / WSCALE)

    phase_b.close()

    # ---------------- Phase D: scores -> exp -> xbar eT -> C1 -------------
    # Software-pipelined by TWO s-blocks: while the PE runs scores(j) and
    # C1(j-2), block j-1 flows through exp (ACT) -> xbar transpose (SP DMA)
    # -> de-interleave (Pool) with ~7us of slack before C1(j-1) needs it.
    carry = {}

    def scores_mms(j):
        ps = [pp6.tile([P, 512], FP32, tag="pp6", name=f"pp6_{qi}")
              for qi in range(4)]
        for ecp in range(4):
            for tq in range(4):
                nc.tensor.matmul(
                    ps[tq],
                    lhsT=LT[:, 2 * ecp:2 * ecp + 2, j * P:(j + 1) * P],
                    rhs=RT[:, 2 * ecp:2 * ecp + 2, tq * 512:(tq + 1) * 512],
                    start=(ecp == 0), stop=(ecp == 3), perf_mode=DR)
        return ps

    rsparts = {}

    def scores_exp(j, ps):
        # accum_out gives the per-quadrant row sums for free on the ACT
        # accumulator (a full-row DVE reduce costs 2.3us - too much)
        rs_part = stats.tile([P, 4], FP32, tag="rsp")
        for tq in range(4):
            nc.scalar.activation(
                out=e_full[:, j, tq * 512:(tq + 1) * 512], in_=ps[tq],
                func=mybir.ActivationFunctionType.Exp, scale=SCALE,
                accum_out=rs_part[:, tq:tq + 1])
        rsparts[j] = rs_part

    def rowsum_recip(j):
        rowsum = stats.tile([P, 1], FP32, tag="rs")
        nc.vector.reduce_sum(out=rowsum, in_=rsparts.pop(j),
                             axis=mybir.AxisListType.X)
        rrec = stats.tile([P, 1], FP32, tag="rrec")
        nc.vector.reciprocal(out=rrec, in_=rowsum)
        carry[j] = rrec

    def transposes(j):
        # e_full[:, j, :] viewed as uint16 pairs -> packed transposed tile
        # eT16[p, uc, s]: the uint16 at (p, uc, s) is the fp8 pair
        # (e[s, 256uc+2p], e[s, 256uc+2p+1]). Two halves so the first can
        # launch as soon as the first two exp quadrants are done.
        eT16 = eTpk.tile([P, ND, P], U16, tag="eT16")
        e16 = e_full[:, j, :].bitcast(U16)
        nc.sync.dma_start_transpose(out=eT16[:, 0:ND // 2, :],
                                    in_=e16[:, 0:S // 4])
        nc.sync.dma_start_transpose(out=eT16[:, ND // 2:ND, :],
                                    in_=e16[:, S // 4:S // 2])
        return eT16

    def deint(j, eT16):
        # Byte de-interleave into the standard DoubleRow stationary layout:
        # eT_panel[p, uc, b, s] = e[s, 256uc+2p+b]. One half on ACT, one on
        # DVE (~1.1us each): the Pool engine runs this strided byte copy ~5x
        # slower, and putting both on ACT made ACT the critical engine.
        eT_panel = eTp.tile([P, ND, 2, P], FP8, tag="eTp")
        for h, eng in ((0, nc.scalar), (1, nc.vector)):
            sl = slice(h * (ND // 2), (h + 1) * (ND // 2))
            src = eT16[:, sl, :].bitcast(FP8).rearrange(
                "p uc (s two) -> p uc two s", two=2)
            if eng is nc.scalar:
                eng.copy(out=eT_panel[:, sl, :, :], in_=src)
            else:
                eng.tensor_copy(out=eT_panel[:, sl, :, :], in_=src)
        return eT_panel

    def ctx1_mms(i, eT_panel):
        rrec = carry.pop(i)
        # C1: ctx_l[i-block] = (e @ rhs) * rrec; uc-outer so the two q psums
        # share each stationary load
        qs = [pp_c1.tile([P, 512], FP32, tag="c1", name=f"c1_{qi}")
              for qi in range(2)]
        for uc in range(S // 256):
            for q in range(2):
                nc.tensor.matmul(
                    qs[q],
                    lhsT=eT_panel[:, uc, :, :],
                    rhs=rhs_nat2[:, uc, :, q * 512:(q + 1) * 512],
                    start=(uc == 0), stop=(uc == S // 256 - 1), perf_mode=DR)
        osb = outp.tile([P, D], FP32, tag="osb")
        for q in range(2):
            nc.vector.tensor_scalar_mul(
                out=osb[:, q * 512:(q + 1) * 512], in0=qs[q], scalar1=rrec)
            nc.sync.dma_start(
                out=ctx_l[i * P:(i + 1) * P, q * 512:(q + 1) * 512],
                in_=osb[:, q * 512:(q + 1) * 512])

    def csacc_reduce(i, eT_panel):
        # partial column sums on the DVE, issued last in the DVE queue so
        # they fill idle time; out csacc[p, b, uc, i]
        nc.vector.reduce_sum(
            out=csacc[:, :, :, i:i + 1],
            in_=eT_panel[:].rearrange("p uc two s -> p two uc s"),
            axis=mybir.AxisListType.X)

    pipe = {}
    tpk = {}
    for j in range(NS + 2):
        if j < NS:
            ps = scores_mms(j)
        if 1 <= j <= NS:
            # ACT: de-interleave block j-1 first (inputs landed last iter),
            # so the copies never make the ACT queue idle-wait mid-iteration
            pipe[j - 1] = deint(j - 1, tpk.pop(j - 1))
        if 1 <= j <= NS:
            # DVE: independent work first (row sums of j-1, its column-sum
            # partials), the PE-gated ts_muls of j-2 last
            rowsum_recip(j - 1)
        if j < NS:
            scores_exp(j, ps)
            tpk[j] = transposes(j)
        if 1 <= j <= NS:
            csacc_reduce(j - 1, pipe[j - 1])
        if j >= 2:
            ctx1_mms(j - 2, pipe.pop(j - 2))

    # ---------------- D->E boundary: straighten the column sums -----------
    # csum_t[p, b, uc] = sum over all 16 blocks of csacc; then four tiny
    # shuffle DMAs map (p, b) -> partition p'=2p+b for even (p<64) / odd
    # (p>=64) t-blocks: colsum_ev[p', uc] = column sum for t-block 2uc at
    # t%128 = p'. Ready ~2us into phase E; the first ts_mul consumer has
    # ~10us of phase-E pipeline ahead of it.
    nc.vector.reduce_sum(out=csum_t[:], in_=csacc[:],
                         axis=mybir.AxisListType.X)
    for dst, p0 in ((colsum_ev, 0), (colsum_od, P // 2)):
        half = dst[:].rearrange("(p two) uc -> p two uc", two=2)
        for b in range(2):
            nc.gpsimd.dma_start(out=half[:, b, :],
                                in_=csum_t[p0:p0 + P // 2, b, :, 0])

    # ---------------- Phase E: C2 (column softmax context) ----------------
    # The last t-block runs q-major so its first half can drain while the
    # second half multiplies.
    for tb in range(NS):
        qs = [pp6.tile([P, 512], FP32, tag="pp6", name=f"pp6_{qi}")
              for qi in range(2)]
        if tb < NS - 1:
            for scp in range(ND):
                lw = e_full[:, 2 * scp:2 * scp + 2, tb * P:(tb + 1) * P]
                for q in range(2):
                    nc.tensor.matmul(
                        qs[q], lhsT=lw,
                        rhs=lhs_nat[:, 2 * scp:2 * scp + 2,
                                    q * 512:(q + 1) * 512],
                        start=(scp == 0), stop=(scp == ND - 1), perf_mode=DR)
        else:
            for q in range(2):
                for scp in range(ND):
                    nc.tensor.matmul(
                        qs[q],
                        lhsT=e_full[:, 2 * scp:2 * scp + 2,
                                    tb * P:(tb + 1) * P],
                        rhs=lhs_nat[:, 2 * scp:2 * scp + 2,
                                    q * 512:(q + 1) * 512],
                        start=(scp == 0), stop=(scp == ND - 1), perf_mode=DR)
        colsum_part = colsum_ev if tb % 2 == 0 else colsum_od
        crec = stats.tile([P, 1], FP32, tag="crec")
        nc.vector.reciprocal(out=crec,
                             in_=colsum_part[:, tb // 2:tb // 2 + 1])
        osb = outp.tile([P, D], FP32, tag="osb")
        for q in range(2):
            nc.vector.tensor_scalar_mul(
                out=osb[:, q * 512:(q + 1) * 512], in0=qs[q], scalar1=crec)
            nc.sync.dma_start(
                out=ctx_r[tb * P:(tb + 1) * P, q * 512:(q + 1) * 512],
                in_=osb[:, q * 512:(q + 1) * 512])


def build_bass():
    nc = bacc.Bacc()
    lhsT8 = nc.declare_dram_parameter("lhsT8", [D, S], FP8, isOutput=False)
    rhsT8 = nc.declare_dram_parameter("rhsT8", [D, S], FP8, isOutput=False)
    lhs8 = nc.declare_dram_parameter("lhs8", [S, D], FP8, isOutput=False)
    rhs_dr8 = nc.declare_dram_parameter("rhs_dr8", [P, S // 256, 2, D], FP8,
                                        isOutput=False)
    wl8 = nc.declare_dram_parameter("wl8", [D, D], FP8, isOutput=False)
    wr8 = nc.declare_dram_parameter("wr8", [D, D], FP8, isOutput=False)
    ctx_l = nc.declare_dram_parameter("ctx_l", [S, D], FP32, isOutput=True)
    ctx_r = nc.declare_dram_parameter("ctx_r", [S, D], FP32, isOutput=True)
    with tile.TileContext(nc) as tc:
        with ExitStack() as ctx:
            _build_body(ctx, tc, lhsT8[:], rhsT8[:], lhs8[:], rhs_dr8[:],
                        wl8[:], wr8[:], ctx_l[:], ctx_r[:])
    nc.compile()
    return nc


def _profiled_run(nc, in_maps):
    """Run via PJRT with NTFF profiling of core 0; returns (results, info)."""
    import glob
    import tempfile

    from concourse import bass2jax

    try:
        from trn_agent_boot.trn_boot import _ntff_profile_via_ctypes
        hook = _ntff_profile_via_ctypes("/opt/axon/libaxon_pjrt.so")
    except Exception as e:
        print(f"[kernel] NTFF hook unavailable ({e}); running untraced",
              file=sys.stderr)
        hook = None
    if hook is None:
        return bass2jax.run_bass_via_pjrt(nc, in_maps, n_cores=N_CORES), None

    tmpdir = tempfile.mkdtemp(prefix="bass_ntff_")
    with hook(tmpdir, [0]):
        results = bass2jax.run_bass_via_pjrt(nc, in_maps, n_cores=N_CORES)

    ntffs = glob.glob(os.path.join(tmpdir, "*_body*.ntff"))
    if not ntffs:
        print(f"[kernel] no NTFFs in {tmpdir}: {os.listdir(tmpdir)}",
              file=sys.stderr)
        return results, None
    import gauge.profiler
    from concourse._compat import FishPath

    profile = gauge.profiler.Profile(
        profile_path=FishPath(tmpdir),
        kernel_dev_mode=True,
        profile_on_exit=False,
        bass_kernel=nc.m,
        offline_processing=True,
        fname="*_body*",
    )
    try:
        pres = profile.to_perfetto(model_index=(0,))
        if pres:
            return results, (pres[0].exec_time_ns, pres[0].trace_path, tmpdir,
                             pres[0].insts)
    except Exception as e:
        print(f"[kernel] perfetto conversion failed: {e}", file=sys.stderr)
    return results, None


def kernel(lhs, rhs, W_lhs, W_rhs):
    """Full inputs in, full outputs out. Shards batch across 8 cores."""
    global last_exec_time_ns, last_results
    from concourse import bass2jax

    f8 = ml_dtypes.float8_e4m3
    lhs = np.ascontiguousarray(np.asarray(lhs, dtype=np.float32))
    rhs = np.ascontiguousarray(np.asarray(rhs, dtype=np.float32))
    lhs8 = lhs.astype(f8)
    rhs8 = rhs.astype(f8)
    lhsT8 = np.ascontiguousarray(lhs.transpose(0, 2, 1)).astype(f8)
    rhsT8 = np.ascontiguousarray(rhs.transpose(0, 2, 1)).astype(f8)
    # interleaved DoubleRow layout for C1's moving operand:
    # rhs_dr8[n, p, c, b, d] = rhs[n, 256c + 2p + b, d]
    rhs_dr8 = np.ascontiguousarray(
        rhs8.reshape(rhs8.shape[0], S // 256, P, 2, D).transpose(0, 2, 1, 3, 4))
    wl8 = np.ascontiguousarray(
        np.asarray(W_lhs, dtype=np.float32).T * WSCALE).astype(f8)
    wr8 = np.ascontiguousarray(
        np.asarray(W_rhs, dtype=np.float32).T * WSCALE).astype(f8)

    nc = build_bass()
    in_maps = [
        {"lhsT8": lhsT8[i], "rhsT8": rhsT8[i], "lhs8": lhs8[i],
         "rhs_dr8": rhs_dr8[i], "wl8": wl8, "wr8": wr8}
        for i in range(N_CORES)
    ]
    if os.environ.get("KERNEL_TRACE", "0") == "1":
        results, info = _profiled_run(nc, in_maps)
        if info is not None:
            last_exec_time_ns = info[0]
            last_results = info
    else:
        results = bass2jax.run_bass_via_pjrt(nc, in_maps, n_cores=N_CORES)
    ctx_l = np.stack([np.asarray(results[i]["ctx_l"]) for i in range(N_CORES)])
    ctx_r = np.stack([np.asarray(results[i]["ctx_r"]) for i in range(N_CORES)])
    out_lhs = np.concatenate([lhs, ctx_l], axis=2)
    out_rhs = np.concatenate([rhs, ctx_r], axis=2)
    return out_lhs, out_rhs
